# revision 1
# baseline (speedup 1.0000x reference)
"""Causal self-attention (RoPE + RMS-norm QK, 16 heads) on 8 Trainium2 cores.

Sharding: core c = (b, g) with b = c // 4 (batch), g = c % 4 (head group of 4).
Each core computes q/k/v projections for its 4 heads from x[b], runs causal
attention, and the out-projection restricted to its head-group columns of
wproj; the host sums the 4 partial outputs per batch.

Per-core layout ("transposed-S"): projections produce Q^T/K^T with head-dim
on partitions (the natural layout for the QK^T contraction), V in natural
[t, d] layout (the natural lhsT for P·V). Scores are computed transposed
(S^T[tk, tq]) so softmax needs no transposes: exp only (logits are bounded
by sqrt(D) after RMS-norm, so max-subtraction is unnecessary), the
denominator comes from an all-ones lhsT matmul that broadcasts column sums
across partitions, and the divide happens on the P·V result's move out of
PSUM. The out-projection is fused per tq-chunk. All heavy matmuls run in
fp32r (hw-rounded fp32, ~1.5e-4 rel err, 4x the fp32 PE rate).
"""

import numpy as np

import concourse.bass as bass
import concourse.mybir as mybir
import concourse.tile as tile
from concourse import bacc
from concourse.bass_utils import run_bass_kernel_spmd

P = 128          # partitions / head dim
T = 2048         # sequence length
C = 2048         # model dim
HL = 4           # heads per core
DL = HL * P      # local projection width (512)
NCO = C // P     # c-chunks (16)
XCH = 512        # x t-chunk width for projections
NXCH = T // XCH  # 8
QCH = 512        # tq-chunk width for attention
NQCH = T // QCH  # 4
NTT = T // P     # t-tiles (16)

F32 = mybir.dt.float32
F32R = mybir.dt.float32r
MUL = mybir.AluOpType.mult


def build_program():
    nc = bacc.Bacc("TRN2", target_bir_lowering=False, debug=False, num_devices=8)

    xT = nc.dram_tensor("xT", [C, T], F32R, kind="ExternalInput")
    wqT = nc.dram_tensor("wqT", [C, DL], F32R, kind="ExternalInput")
    wkT = nc.dram_tensor("wkT", [C, DL], F32R, kind="ExternalInput")
    wvT = nc.dram_tensor("wvT", [C, DL], F32R, kind="ExternalInput")
    wpT = nc.dram_tensor("wpT", [DL, C], F32R, kind="ExternalInput")
    csA_d = nc.dram_tensor("csA", [P, T], F32, kind="ExternalInput")   # cos|cos
    csB_d = nc.dram_tensor("csB", [P, T], F32, kind="ExternalInput")   # sin|sin
    tri_d = nc.dram_tensor("tri", [P, 4, QCH], F32, kind="ExternalInput")
    ones_d = nc.dram_tensor("ones", [P, P], F32R, kind="ExternalInput")
    out_p = nc.dram_tensor("out_p", [T, C], F32, kind="ExternalOutput")

    xT_r = xT.ap().rearrange("(co p) t -> p co t", p=P)

    with tile.TileContext(nc) as tc:
        with (
            tc.tile_pool(name="base", bufs=1) as base,
            tc.tile_pool(name="ps_acc", bufs=3, space="PSUM") as ps_acc,
            tc.tile_pool(name="ps_st", bufs=2, space="PSUM") as ps_st,
            tc.tile_pool(name="ps_ot", bufs=2, space="PSUM") as ps_ot,
            tc.tile_pool(name="ps_den", bufs=1, space="PSUM") as ps_den,
        ):
            QT_sb = base.tile([P, HL, T], F32R, tag="QT")   # [d, h, tq]
            KT_sb = base.tile([P, HL, T], F32R, tag="KT")   # [d, h, tk]
            ones_sb = base.tile([P, P], F32R, tag="ones")

            # ---- phases A/B: Q then K projection + RoPE + RMS-norm ----
            with (
                tc.tile_pool(name="ab", bufs=1) as ab,
                tc.tile_pool(name="abw", bufs=2) as abw,
                tc.tile_pool(name="abw1", bufs=2) as abw1,
                tc.tile_pool(name="abx", bufs=2) as abx,
            ):
                csA_sb = ab.tile([P, T], F32, tag="csA")
                csB_sb = ab.tile([P, T], F32, tag="csB")

                for w_dram, dst_sb, ln_scale, name in (
                    (wqT, QT_sb, 1.0, "q"),
                    (wkT, KT_sb, 1.0 / P, "k"),
                ):
                    w_sb = ab.tile([P, NCO, DL], F32R, tag="w")
                    w_r = w_dram.ap().rearrange("(co p) d -> p co d", p=P)
                    for c in range(NCO):
                        nc.sync.dma_start(w_sb[:, c, :], w_r[:, c, :])
                    if name == "q":
                        # consts ride behind the first weight chunks
                        nc.sync.dma_start(ones_sb[:], ones_d.ap())
                        nc.sync.dma_start(csA_sb[:], csA_d.ap())
                        nc.sync.dma_start(csB_sb[:], csB_d.ap())
                    for tcx in range(NXCH):
                        cols = slice(tcx * XCH, (tcx + 1) * XCH)
                        x_sb = abx.tile([P, NCO, XCH], F32R, tag="x")
                        nc.sync.dma_start(x_sb[:], xT_r[:, :, cols])

                        def project(h):
                            psq = ps_acc.tile([P, XCH], F32, tag="acc")
                            for c in range(NCO):
                                nc.tensor.matmul(
                                    psq[:],
                                    w_sb[:, c, h * P : (h + 1) * P],
                                    x_sb[:, c, :],
                                    start=(c == 0),
                                    stop=(c == NCO - 1),
                                )
                            return psq

                        def epilogue(h, psq):
                            # Copy PSUM out once (frees the accumulation bank fast),
                            # then RoPE fully in SBUF. csA = cos|cos, csB = sin|-sin,
                            # so tmp = [-q2*sin | q1*sin] with base-aligned reads and
                            # the combine is one full-height subtract.
                            qc = abw.tile([P, XCH], F32, tag="qc")
                            nc.scalar.copy(qc[:], psq[:])
                            qr = abw.tile([P, XCH], F32, tag="qr")
                            tmp = abw1.tile([P, XCH], F32, tag="tmp")
                            lo, hi = slice(0, 64), slice(64, P)
                            nc.vector.tensor_tensor(tmp[lo, :], qc[hi, :], csB_sb[hi, cols], MUL)
                            nc.vector.tensor_tensor(tmp[hi, :], qc[lo, :], csB_sb[lo, cols], MUL)
                            nc.vector.tensor_tensor(qr[:], qc[:], csA_sb[:, cols], MUL)
                            nc.vector.tensor_tensor(
                                qr[:], qr[:], tmp[:], mybir.AluOpType.subtract
                            )
                            # RMS: ssq broadcast over partitions via all-ones lhsT;
                            # rsqrt on ACT as exp(-0.5*ln(ssq*ln_scale))
                            q2t = abw1.tile([P, XCH], F32R, tag="q2t")
                            nc.vector.tensor_tensor(q2t[:], qr[:], qr[:], MUL)
                            ssq = ps_st.tile([P, XCH], F32, tag="st")
                            nc.tensor.matmul(ssq[:], ones_sb[:], q2t[:], start=True, stop=True)
                            lnt = abw1.tile([P, XCH], F32, tag="lnt")
                            nc.scalar.activation(
                                lnt[:], ssq[:], mybir.ActivationFunctionType.Ln,
                                scale=ln_scale,
                            )
                            rinv = abw1.tile([P, XCH], F32, tag="rinv")
                            nc.scalar.activation(
                                rinv[:], lnt[:], mybir.ActivationFunctionType.Exp,
                                scale=-0.5,
                            )
                            nc.vector.tensor_tensor(dst_sb[:, h, cols], qr[:], rinv[:], MUL)

                        # software-pipeline the emission: head h's epilogue is
                        # interleaved behind head h+1's projection matmuls
                        prev = None
                        for h in range(HL):
                            psq = project(h)
                            if prev is not None:
                                epilogue(*prev)
                            prev = (h, psq)
                        epilogue(*prev)

            with tc.tile_pool(name="vp", bufs=1) as vp:
                V_sb = vp.tile([P, NTT, DL], F32R, tag="V")   # [t_sub, t_tile, d]

                # ---- phase C: V projection ----------------------------
                with (
                    tc.tile_pool(name="cw", bufs=1) as cw,
                    tc.tile_pool(name="cx", bufs=2) as cx,
                ):
                    wv_sb = cw.tile([P, NCO, DL], F32R, tag="wv")
                    wv_r = wvT.ap().rearrange("(co p) d -> p co d", p=P)
                    for c in range(NCO):
                        nc.sync.dma_start(wv_sb[:, c, :], wv_r[:, c, :])
                    for tcx in range(NXCH):
                        cols = slice(tcx * XCH, (tcx + 1) * XCH)
                        x_sb = cx.tile([P, NCO, XCH], F32R, tag="x")
                        nc.sync.dma_start(x_sb[:], xT_r[:, :, cols])
                        for m in range(XCH // P):
                            psv = ps_acc.tile([P, DL], F32, tag="acc")
                            for c in range(NCO):
                                nc.tensor.matmul(
                                    psv[:],
                                    x_sb[:, c, m * P : (m + 1) * P],
                                    wv_sb[:, c, :],
                                    start=(c == 0),
                                    stop=(c == NCO - 1),
                                )
                            nc.scalar.copy(V_sb[:, tcx * (XCH // P) + m, :], psv[:])

                # ---- phase D: attention + fused out-projection --------
                with (
                    tc.tile_pool(name="dw", bufs=1) as dw,
                    tc.tile_pool(name="de", bufs=6) as de,
                    tc.tile_pool(name="de4", bufs=2) as de4,
                    tc.tile_pool(name="de4f", bufs=1) as de4f,
                    tc.tile_pool(name="dm", bufs=2) as dm,
                ):
                    tri4_sb = dw.tile([P, 4, QCH], F32, tag="tri4")
                    nc.sync.dma_start(tri4_sb[:], tri_d.ap())
                    wp_sb = dw.tile([P, HL, C], F32R, tag="wp")
                    nc.sync.dma_start(
                        wp_sb[:], wpT.ap().rearrange("(h p) j -> p h j", p=P)
                    )
                    NSTR = QCH // P  # diagonal-band tiles per chunk
                    for j in range(NQCH):
                        ot_ch = dm.tile([P, HL, QCH], F32R, tag="otch")
                        ntk = (j + 1) * NSTR
                        for h in range(HL):
                            ot_ps = ps_ot.tile([P, QCH], F32, tag="ot")
                            den_ps = ps_den.tile([P, QCH], F32, tag="den")
                            # e4 holds the 4 diagonal-band tiles; the causal mask is
                            # one contiguous multiply with tri4 (which also zeroes
                            # the never-written leading columns too)
                            e4 = de4.tile([P, NSTR, QCH], F32R, tag="e4")
                            e4f = de4f.tile([P, NSTR, QCH], F32, tag="e4f")
                            for r in range(NSTR):
                                i = j * NSTR + r
                                st = ps_st.tile([P, QCH], F32, tag="st")
                                nc.tensor.matmul(
                                    st[:],
                                    KT_sb[:, h, i * P : (i + 1) * P],
                                    QT_sb[:, h, j * QCH : (j + 1) * QCH],
                                    start=True,
                                    stop=True,
                                )
                                nc.scalar.activation(
                                    e4f[:, r, :], st[:],
                                    mybir.ActivationFunctionType.Exp,
                                )
                            # fp32 x fp32 -> fp32r: keeps the DVE on its fast path
                            nc.vector.tensor_tensor(e4[:], e4f[:], tri4_sb[:], MUL)
                            for i in range(ntk):
                                r = i - j * NSTR
                                if r >= 0:
                                    e_ap = e4[:, r, :]
                                else:
                                    st = ps_st.tile([P, QCH], F32, tag="st")
                                    nc.tensor.matmul(
                                        st[:],
                                        KT_sb[:, h, i * P : (i + 1) * P],
                                        QT_sb[:, h, j * QCH : (j + 1) * QCH],
                                        start=True,
                                        stop=True,
                                    )
                                    e_sb = de.tile([P, QCH], F32R, tag="e")
                                    nc.scalar.activation(
                                        e_sb[:], st[:],
                                        mybir.ActivationFunctionType.Exp,
                                    )
                                    e_ap = e_sb[:]
                                nc.tensor.matmul(
                                    ot_ps[:],
                                    V_sb[:, i, h * P : (h + 1) * P],
                                    e_ap,
                                    start=(i == 0),
                                    stop=(i == ntk - 1),
                                )
                                nc.tensor.matmul(
                                    den_ps[:],
                                    ones_sb[:],
                                    e_ap,
                                    start=(i == 0),
                                    stop=(i == ntk - 1),
                                )
                            recip = dm.tile([P, QCH], F32, tag="recip")
                            dscr = dm.tile([P, QCH], F32, tag="dscr")
                            nc.vector.reciprocal_approx_accurate(recip[:], den_ps[:], dscr[:])
                            nc.vector.tensor_tensor(
                                ot_ch[:, h, :], ot_ps[:], recip[:], MUL
                            )
                        # fused out-projection for tq-chunk j
                        for u in range(QCH // P):
                            for jc in range(NQCH):
                                po = ps_acc.tile([P, QCH], F32, tag="acc")
                                for h in range(HL):
                                    nc.tensor.matmul(
                                        po[:],
                                        ot_ch[:, h, u * P : (u + 1) * P],
                                        wp_sb[:, h, jc * QCH : (jc + 1) * QCH],
                                        start=(h == 0),
                                        stop=(h == HL - 1),
                                    )
                                osb = dm.tile([P, QCH], F32, tag="osb")
                                nc.vector.tensor_copy(osb[:], po[:])
                                nc.sync.dma_start(
                                    out_p.ap()[
                                        j * QCH + u * P : j * QCH + (u + 1) * P,
                                        jc * QCH : (jc + 1) * QCH,
                                    ],
                                    osb[:],
                                )

    nc.compile()
    return nc


_NC = None


def _get_nc():
    global _NC
    if _NC is None:
        _NC = build_program()
    return _NC


def _host_inputs(x, cos, sin, wq, wk, wv, wproj):
    B = x.shape[0]
    cosT = np.ascontiguousarray(cos[0, :, 0, :].T).astype(np.float32)  # [64, T]
    sinT = np.ascontiguousarray(sin[0, :, 0, :].T).astype(np.float32)
    csA = np.concatenate([cosT, cosT], axis=0)
    csB = np.concatenate([sinT, -sinT], axis=0)
    # tri4[p, r, f] = 1 iff causal (tk=128r+p <= tq=f) within a diagonal band
    rr, pp, ff = np.meshgrid(np.arange(4), np.arange(P), np.arange(QCH), indexing="ij")
    tri = np.ascontiguousarray(
        (pp + 128 * rr <= ff).astype(np.float32).transpose(1, 0, 2)
    )
    ones = np.ones((P, P), np.float32)

    xTs = [np.ascontiguousarray(x[b].T) for b in range(B)]
    in_maps = []
    for core in range(8):
        b, g = divmod(core, 4)
        sl = slice(g * DL, (g + 1) * DL)
        in_maps.append({
            "xT": xTs[b],
            "wqT": np.ascontiguousarray(wq[sl, :].T),
            "wkT": np.ascontiguousarray(wk[sl, :].T),
            "wvT": np.ascontiguousarray(wv[sl, :].T),
            "wpT": np.ascontiguousarray(wproj[:, sl].T),
            "csA": csA, "csB": csB, "tri": tri, "ones": ones,
        })
    return in_maps


def kernel(x, cos, sin, wq, wk, wv, wproj, _trace=False):
    nc = _get_nc()
    in_maps = _host_inputs(x, cos, sin, wq, wk, wv, wproj)
    res = run_bass_kernel_spmd(nc, in_maps, core_ids=list(range(8)), trace=_trace)
    parts = [res.results[c]["out_p"].astype(np.float64) for c in range(8)]
    out = np.stack([
        sum(parts[0:4]).astype(np.float32),
        sum(parts[4:8]).astype(np.float32),
    ])
    kernel.last_exec_time_ns = res.exec_time_ns
    kernel.last_result = res
    return out



# revision 2
# speedup vs baseline: 1.2029x; 1.2029x over previous
"""Causal self-attention (RoPE + RMS-norm QK, 16 heads) on 8 Trainium2 cores.

Sharding: core c = (b, g) with b = c // 4 (batch), g = c % 4 (head group of 4).
Each core computes q/k/v projections for its 4 heads from x[b], runs causal
attention, and the out-projection restricted to its head-group columns of
wproj; the host sums the 4 partial outputs per batch.

v2 layout vs the fp32r baseline:
- everything bf16 on the input side (same PE rate as fp32r, half the DMA and
  SBUF traffic, 2-4x DVE throughput); fp32 only inside PSUM accumulation.
- single pass over x: Q, K and V projections for each x t-chunk, so x is
  DMA'd once (bf16) instead of three times (fp32).
- all weights prefetched behind the first chunk's compute; DMAs ordered so
  the first matmul can start after ~2 MB of transfers.
- rms-norm uses DVE reciprocal_approx_fast + ACT Sqrt, so phase P only ever
  needs the sqrt act table and phase D the exp table: 2 table loads total
  (the Ln/Exp alternation in the baseline caused 65 table loads, 83us).
- bf16 output (host sums partials in fp32), halving the output DMA.

Per-core layout ("transposed-S"): projections produce Q^T/K^T with head-dim
on partitions, V in natural [t, d] layout. Scores are computed transposed
(S^T[tk, tq]) so softmax needs no transposes: exp only (logits are bounded
by sqrt(D) after RMS-norm), the denominator comes from an all-ones lhsT
matmul that broadcasts column sums across partitions, and the divide happens
on the P.V result's move out of PSUM. The out-projection is fused per
tq-chunk.
"""

import numpy as np
import ml_dtypes

import concourse.bass as bass
import concourse.mybir as mybir
import concourse.tile as tile
from concourse import bacc
from concourse.bass_utils import run_bass_kernel_spmd

P = 128          # partitions / head dim
T = 2048         # sequence length
C = 2048         # model dim
HL = 4           # heads per core
DL = HL * P      # local projection width (512)
NCO = C // P     # c-chunks (16)
XCH = 512        # x t-chunk width for projections
NXCH = T // XCH  # 4
QCH = 512        # tq-chunk width for attention
NQCH = T // QCH  # 4
NSTR = QCH // P  # diagonal-band tiles per chunk (4)
NTT = T // P     # t-tiles (16)

F32 = mybir.dt.float32
BF16 = mybir.dt.bfloat16
MUL = mybir.AluOpType.mult
SUB = mybir.AluOpType.subtract
SQRT = mybir.ActivationFunctionType.Sqrt
EXP = mybir.ActivationFunctionType.Exp


def build_program():
    nc = bacc.Bacc("TRN2", target_bir_lowering=False, debug=False, num_devices=8)

    xT = nc.dram_tensor("xT", [C, T], BF16, kind="ExternalInput")
    wqT = nc.dram_tensor("wqT", [C, DL], BF16, kind="ExternalInput")
    wkT = nc.dram_tensor("wkT", [C, DL], BF16, kind="ExternalInput")
    wvT = nc.dram_tensor("wvT", [C, DL], BF16, kind="ExternalInput")
    wpT = nc.dram_tensor("wpT", [DL, C], BF16, kind="ExternalInput")
    csA_d = nc.dram_tensor("csA", [P, T], BF16, kind="ExternalInput")   # cos|cos
    csB_d = nc.dram_tensor("csB", [P, T], BF16, kind="ExternalInput")   # sin|-sin
    tri_d = nc.dram_tensor("tri", [P, NSTR, QCH], BF16, kind="ExternalInput")
    ones_d = nc.dram_tensor("ones", [P, P], BF16, kind="ExternalInput")
    out_p = nc.dram_tensor("out_p", [T, C], BF16, kind="ExternalOutput")

    xT_r = xT.ap().rearrange("(co p) t -> p co t", p=P)

    with tile.TileContext(nc) as tc:
        with tc.tile_pool(name="base", bufs=1) as base:
            QT_sb = base.tile([P, HL, T], BF16, tag="QT")   # [d, h, tq]
            KT_sb = base.tile([P, HL, T], BF16, tag="KT")   # [d, h, tk]
            V_sb = base.tile([P, NTT, DL], BF16, tag="V")   # [t_sub, t_tile, d]
            ones_sb = base.tile([P, P], BF16, tag="ones")
            csA_sb = base.tile([P, T], BF16, tag="csA")
            csB_sb = base.tile([P, T], BF16, tag="csB")
            wp_sb = base.tile([P, HL, C], BF16, tag="wp")
            tri4_sb = base.tile([P, NSTR, QCH], BF16, tag="tri4")

            # ---- phase P: Q/K/V projections in one pass over x --------
            with (
                tc.tile_pool(name="pw", bufs=1) as pw,
                tc.tile_pool(name="px", bufs=2) as px,
                tc.tile_pool(name="pe1", bufs=2) as pe1,
                tc.tile_pool(name="pe2", bufs=2) as pe2,
                tc.tile_pool(name="ps_acc", bufs=3, space="PSUM") as ps_acc,
                tc.tile_pool(name="ps_ssq", bufs=2, space="PSUM") as ps_ssq,
            ):
                wq_sb = pw.tile([P, NCO, DL], BF16, tag="wq")
                wk_sb = pw.tile([P, NCO, DL], BF16, tag="wk")
                wv_sb = pw.tile([P, NCO, DL], BF16, tag="wv")
                wq_r = wqT.ap().rearrange("(co p) d -> p co d", p=P)
                wk_r = wkT.ap().rearrange("(co p) d -> p co d", p=P)
                wv_r = wvT.ap().rearrange("(co p) d -> p co d", p=P)

                def project_qk(x_sb, w_sb, h):
                    psq = ps_acc.tile([P, XCH], F32, tag="acc")
                    for c in range(NCO):
                        nc.tensor.matmul(
                            psq[:],
                            w_sb[:, c, h * P : (h + 1) * P],
                            x_sb[:, c, :],
                            start=(c == 0),
                            stop=(c == NCO - 1),
                        )
                    return psq

                def project_v(x_sb, m):
                    psv = ps_acc.tile([P, DL], F32, tag="acc")
                    for c in range(NCO):
                        nc.tensor.matmul(
                            psv[:],
                            x_sb[:, c, m * P : (m + 1) * P],
                            wv_sb[:, c, :],
                            start=(c == 0),
                            stop=(c == NCO - 1),
                        )
                    return psv

                def epilogue_qk(cols, dst_sb, h, scale, psq):
                    # RoPE fully in bf16 SBUF. csA = cos|cos, csB = sin|-sin,
                    # so tmp = [-q2*sin | q1*sin] with base-aligned reads and
                    # the combine is one full-height subtract.
                    qc = pe1.tile([P, XCH], BF16, tag="qc")
                    nc.scalar.copy(qc[:], psq[:])
                    tmp = pe2.tile([P, XCH], BF16, tag="tmp")
                    lo, hi = slice(0, 64), slice(64, P)
                    nc.vector.tensor_tensor(tmp[lo, :], qc[hi, :], csB_sb[hi, cols], MUL)
                    nc.vector.tensor_tensor(tmp[hi, :], qc[lo, :], csB_sb[lo, cols], MUL)
                    qr = pe1.tile([P, XCH], BF16, tag="qr")
                    nc.vector.tensor_tensor(qr[:], qc[:], csA_sb[:, cols], MUL)
                    nc.vector.tensor_tensor(qr[:], qr[:], tmp[:], SUB)
                    # RMS: ssq broadcast over partitions via all-ones lhsT;
                    # rinv = sqrt(scale / ssq) via DVE recip + ACT sqrt
                    q2t = pe2.tile([P, XCH], BF16, tag="q2t")
                    nc.vector.tensor_tensor(q2t[:], qr[:], qr[:], MUL)
                    ssq = ps_ssq.tile([P, XCH], F32, tag="ssq")
                    nc.tensor.matmul(ssq[:], ones_sb[:], q2t[:], start=True, stop=True)
                    r1 = pe2.tile([P, XCH], F32, tag="r1")
                    nc.vector.reciprocal_approx_fast(r1[:], ssq[:])
                    rinv = pe2.tile([P, XCH], BF16, tag="rinv")
                    nc.scalar.activation(rinv[:], r1[:], SQRT, scale=scale)
                    nc.vector.tensor_tensor(dst_sb[:, h, cols], qr[:], rinv[:], MUL)

                def epilogue_v(tcx, m, psv):
                    nc.scalar.copy(V_sb[:, tcx * (XCH // P) + m, :], psv[:])

                for tcx in range(NXCH):
                    cols = slice(tcx * XCH, (tcx + 1) * XCH)
                    x_sb = px.tile([P, NCO, XCH], BF16, tag="x")
                    if tcx == 0:
                        # interleave first-chunk DMAs so the first matmuls
                        # (wq head 0 x chunk 0, low c) start early
                        nc.sync.dma_start(wq_sb[:, 0:8, :], wq_r[:, 0:8, :])
                        nc.sync.dma_start(x_sb[:, 0:8, :], xT_r[:, 0:8, cols])
                        nc.sync.dma_start(wq_sb[:, 8:, :], wq_r[:, 8:, :])
                        nc.sync.dma_start(x_sb[:, 8:, :], xT_r[:, 8:, cols])
                        nc.sync.dma_start(ones_sb[:], ones_d.ap())
                        nc.sync.dma_start(csA_sb[:], csA_d.ap())
                        nc.sync.dma_start(csB_sb[:], csB_d.ap())
                        nc.sync.dma_start(wk_sb[:], wk_r[:])
                        nc.sync.dma_start(wv_sb[:], wv_r[:])
                        nc.sync.dma_start(
                            wp_sb[:], wpT.ap().rearrange("(h p) j -> p h j", p=P)
                        )
                        nc.sync.dma_start(tri4_sb[:], tri_d.ap())
                    else:
                        nc.sync.dma_start(x_sb[:], xT_r[:, :, cols])

                    # software-pipeline: unit u's epilogue is interleaved
                    # behind unit u+1's projection matmuls
                    units = (
                        [("q", h) for h in range(HL)]
                        + [("k", h) for h in range(HL)]
                        + [("v", m) for m in range(XCH // P)]
                    )
                    prev = None
                    for kind, idx in units:
                        if kind == "q":
                            ps = project_qk(x_sb, wq_sb, idx)
                        elif kind == "k":
                            ps = project_qk(x_sb, wk_sb, idx)
                        else:
                            ps = project_v(x_sb, idx)
                        if prev is not None:
                            pkind, pidx, pps = prev
                            if pkind == "q":
                                epilogue_qk(cols, QT_sb, pidx, 1.0, pps)
                            elif pkind == "k":
                                epilogue_qk(cols, KT_sb, pidx, float(P), pps)
                            else:
                                epilogue_v(tcx, pidx, pps)
                        prev = (kind, idx, ps)
                    pkind, pidx, pps = prev
                    if pkind == "v":
                        epilogue_v(tcx, pidx, pps)

            # ---- phase D: attention + fused out-projection ------------
            with (
                tc.tile_pool(name="de", bufs=6) as de,
                tc.tile_pool(name="de4", bufs=2) as de4,
                tc.tile_pool(name="de4f", bufs=2) as de4f,
                tc.tile_pool(name="dm", bufs=2) as dm,
                tc.tile_pool(name="ps_st", bufs=3, space="PSUM") as ps_st,
                tc.tile_pool(name="ps_ot", bufs=2, space="PSUM") as ps_ot,
                tc.tile_pool(name="ps_den", bufs=1, space="PSUM") as ps_den,
                tc.tile_pool(name="ps_po", bufs=2, space="PSUM") as ps_po,
            ):
                for j in range(NQCH):
                    ot_ch = dm.tile([P, HL, QCH], BF16, tag="otch")
                    jq = slice(j * QCH, (j + 1) * QCH)
                    ntk = (j + 1) * NSTR
                    for h in range(HL):
                        ot_ps = ps_ot.tile([P, QCH], F32, tag="ot")
                        den_ps = ps_den.tile([P, QCH], F32, tag="den")
                        # e4 holds the 4 diagonal-band tiles; the causal mask
                        # is one contiguous multiply with tri4 (which also
                        # zeroes the never-written leading columns)
                        e4 = de4.tile([P, NSTR, QCH], BF16, tag="e4")
                        e4f = de4f.tile([P, NSTR, QCH], BF16, tag="e4f")
                        for r in range(NSTR):
                            i = j * NSTR + r
                            st = ps_st.tile([P, QCH], F32, tag="st")
                            nc.tensor.matmul(
                                st[:],
                                KT_sb[:, h, i * P : (i + 1) * P],
                                QT_sb[:, h, jq],
                                start=True,
                                stop=True,
                            )
                            nc.scalar.activation(e4f[:, r, :], st[:], EXP)
                        nc.vector.tensor_tensor(e4[:], e4f[:], tri4_sb[:], MUL)
                        for i in range(ntk):
                            r = i - j * NSTR
                            if r >= 0:
                                e_ap = e4[:, r, :]
                            else:
                                st = ps_st.tile([P, QCH], F32, tag="st")
                                nc.tensor.matmul(
                                    st[:],
                                    KT_sb[:, h, i * P : (i + 1) * P],
                                    QT_sb[:, h, jq],
                                    start=True,
                                    stop=True,
                                )
                                e_sb = de.tile([P, QCH], BF16, tag="e")
                                nc.scalar.activation(e_sb[:], st[:], EXP)
                                e_ap = e_sb[:]
                            nc.tensor.matmul(
                                ot_ps[:],
                                V_sb[:, i, h * P : (h + 1) * P],
                                e_ap,
                                start=(i == 0),
                                stop=(i == ntk - 1),
                            )
                            nc.tensor.matmul(
                                den_ps[:],
                                ones_sb[:],
                                e_ap,
                                start=(i == 0),
                                stop=(i == ntk - 1),
                            )
                        recip = dm.tile([P, QCH], F32, tag="recip")
                        nc.vector.reciprocal_approx_fast(recip[:], den_ps[:])
                        nc.vector.tensor_tensor(
                            ot_ch[:, h, :], ot_ps[:], recip[:], MUL
                        )
                    # fused out-projection for tq-chunk j
                    for u in range(QCH // P):
                        for jc in range(NQCH):
                            po = ps_po.tile([P, QCH], F32, tag="po")
                            for h in range(HL):
                                nc.tensor.matmul(
                                    po[:],
                                    ot_ch[:, h, u * P : (u + 1) * P],
                                    wp_sb[:, h, jc * QCH : (jc + 1) * QCH],
                                    start=(h == 0),
                                    stop=(h == HL - 1),
                                )
                            osb = dm.tile([P, QCH], BF16, tag="osb")
                            # alternate the PSUM-drain between DVE and ACT
                            if (u + jc) % 2 == 0:
                                nc.vector.tensor_copy(osb[:], po[:])
                            else:
                                nc.scalar.copy(osb[:], po[:])
                            nc.sync.dma_start(
                                out_p.ap()[
                                    j * QCH + u * P : j * QCH + (u + 1) * P,
                                    jc * QCH : (jc + 1) * QCH,
                                ],
                                osb[:],
                            )

    nc.compile()
    return nc


_NC = None


def _get_nc():
    global _NC
    if _NC is None:
        _NC = build_program()
    return _NC


def _host_inputs(x, cos, sin, wq, wk, wv, wproj):
    BF = ml_dtypes.bfloat16
    B = x.shape[0]
    cosT = np.ascontiguousarray(cos[0, :, 0, :].T).astype(np.float32)  # [64, T]
    sinT = np.ascontiguousarray(sin[0, :, 0, :].T).astype(np.float32)
    csA = np.concatenate([cosT, cosT], axis=0).astype(BF)
    csB = np.concatenate([sinT, -sinT], axis=0).astype(BF)
    # tri[p, r, f] = 1 iff causal (tk=128r+p <= tq=f) within a diagonal band
    rr, pp, ff = np.meshgrid(np.arange(NSTR), np.arange(P), np.arange(QCH), indexing="ij")
    tri = np.ascontiguousarray(
        (pp + 128 * rr <= ff).astype(np.float32).transpose(1, 0, 2)
    ).astype(BF)
    ones = np.ones((P, P), BF)

    xTs = [np.ascontiguousarray(x[b].T).astype(BF) for b in range(B)]
    in_maps = []
    for core in range(8):
        b, g = divmod(core, 4)
        sl = slice(g * DL, (g + 1) * DL)
        in_maps.append({
            "xT": xTs[b],
            "wqT": np.ascontiguousarray(wq[sl, :].T).astype(BF),
            "wkT": np.ascontiguousarray(wk[sl, :].T).astype(BF),
            "wvT": np.ascontiguousarray(wv[sl, :].T).astype(BF),
            "wpT": np.ascontiguousarray(wproj[:, sl].T).astype(BF),
            "csA": csA, "csB": csB, "tri": tri, "ones": ones,
        })
    return in_maps


def kernel(x, cos, sin, wq, wk, wv, wproj, _trace=False):
    nc = _get_nc()
    in_maps = _host_inputs(x, cos, sin, wq, wk, wv, wproj)
    res = run_bass_kernel_spmd(nc, in_maps, core_ids=list(range(8)), trace=_trace)
    parts = [res.results[c]["out_p"].astype(np.float32) for c in range(8)]
    out = np.stack([
        sum(parts[0:4]),
        sum(parts[4:8]),
    ]).astype(np.float32)
    kernel.last_exec_time_ns = res.exec_time_ns
    kernel.last_result = res
    return out


# revision 10
# speedup vs baseline: 1.2787x; 1.0631x over previous
"""Causal self-attention (RoPE + RMS-norm QK, 16 heads) on 8 Trainium2 cores.

Sharding: core c = (b, g) with b = c // 4 (batch), g = c % 4 (head group of 4).
Each core computes q/k/v projections for its 4 heads from x[b], runs causal
attention, and the out-projection restricted to its head-group columns of
wproj; the host sums the 4 partial outputs per batch.

v2 layout vs the fp32r baseline:
- everything bf16 on the input side (same PE rate as fp32r, half the DMA and
  SBUF traffic, 2-4x DVE throughput); fp32 only inside PSUM accumulation.
- single pass over x: Q, K and V projections for each x t-chunk, so x is
  DMA'd once (bf16) instead of three times (fp32).
- all weights prefetched behind the first chunk's compute; DMAs ordered so
  the first matmul can start after ~2 MB of transfers.
- rms-norm uses DVE reciprocal_approx_fast + ACT Sqrt, so phase P only ever
  needs the sqrt act table and phase D the exp table: 2 table loads total
  (the Ln/Exp alternation in the baseline caused 65 table loads, 83us).
- bf16 output (host sums partials in fp32), halving the output DMA.

Per-core layout ("transposed-S"): projections produce Q^T/K^T with head-dim
on partitions, V in natural [t, d] layout. Scores are computed transposed
(S^T[tk, tq]) so softmax needs no transposes: exp only (logits are bounded
by sqrt(D) after RMS-norm), the denominator comes from an all-ones lhsT
matmul that broadcasts column sums across partitions, and the divide happens
on the P.V result's move out of PSUM. The out-projection is fused per
tq-chunk.
"""

import numpy as np
import ml_dtypes

import concourse.bass as bass
import concourse.mybir as mybir
import concourse.tile as tile
from concourse import bacc
from concourse.bass_utils import run_bass_kernel_spmd

P = 128          # partitions / head dim
T = 2048         # sequence length
C = 2048         # model dim
HL = 4           # heads per core
DL = HL * P      # local projection width (512)
NCO = C // P     # c-chunks (16)
XCH = 512        # x t-chunk width for projections
NXCH = T // XCH  # 4
QCH = 512        # tq-chunk width for attention
NQCH = T // QCH  # 4
NSTR = QCH // P  # diagonal-band tiles per chunk (4)
NTT = T // P     # t-tiles (16)

F32 = mybir.dt.float32
BF16 = mybir.dt.bfloat16
MUL = mybir.AluOpType.mult
SUB = mybir.AluOpType.subtract
SQRT = mybir.ActivationFunctionType.Sqrt
EXP = mybir.ActivationFunctionType.Exp


def build_program():
    nc = bacc.Bacc("TRN2", target_bir_lowering=False, debug=False, num_devices=8)

    xT = nc.dram_tensor("xT", [C, T], BF16, kind="ExternalInput")
    wqT = nc.dram_tensor("wqT", [C, DL], BF16, kind="ExternalInput")
    wkT = nc.dram_tensor("wkT", [C, DL], BF16, kind="ExternalInput")
    wvT = nc.dram_tensor("wvT", [C, DL], BF16, kind="ExternalInput")
    wpT = nc.dram_tensor("wpT", [DL, C], BF16, kind="ExternalInput")
    csA_d = nc.dram_tensor("csA", [P, T], BF16, kind="ExternalInput")   # cos|cos
    csB_d = nc.dram_tensor("csB", [P, T], BF16, kind="ExternalInput")   # sin|-sin
    tri_d = nc.dram_tensor("tri", [P, NSTR, QCH], BF16, kind="ExternalInput")
    ones_d = nc.dram_tensor("ones", [P, P], BF16, kind="ExternalInput")
    ones32_d = nc.dram_tensor("ones32", [P, P], mybir.dt.float32r, kind="ExternalInput")
    out_p = nc.dram_tensor("out_p", [T, C], BF16, kind="ExternalOutput")

    xT_r = xT.ap().rearrange("(co p) t -> p co t", p=P)

    with tile.TileContext(nc) as tc:
        with tc.tile_pool(name="base", bufs=1) as base:
            QT_sb = base.tile([P, HL, T], BF16, tag="QT")   # [d, h, tq]
            KT_sb = base.tile([P, HL, T], BF16, tag="KT")   # [d, h, tk]
            V_sb = base.tile([P, NTT, DL], BF16, tag="V")   # [t_sub, t_tile, d]
            ones_sb = base.tile([P, P], BF16, tag="ones")
            csA_sb = base.tile([P, T], BF16, tag="csA")
            csB_sb = base.tile([P, T], BF16, tag="csB")
            wp_sb = base.tile([P, HL, C], BF16, tag="wp")
            tri4_sb = base.tile([P, NSTR, QCH], BF16, tag="tri4")

            # ---- phase P: Q/K/V projections in one pass over x --------
            with (
                tc.tile_pool(name="pw", bufs=1) as pw,
                tc.tile_pool(name="px", bufs=2) as px,
                tc.tile_pool(name="pe1", bufs=2) as pe1,
                tc.tile_pool(name="pe2", bufs=2) as pe2,
                tc.tile_pool(name="ps_acc", bufs=3, space="PSUM") as ps_acc,
                tc.tile_pool(name="ps_ssq", bufs=2, space="PSUM") as ps_ssq,
            ):
                wq_sb = pw.tile([P, NCO, DL], BF16, tag="wq")
                wk_sb = pw.tile([P, NCO, DL], BF16, tag="wk")
                wv_sb = pw.tile([P, NCO, DL], BF16, tag="wv")
                wq_r = wqT.ap().rearrange("(co p) d -> p co d", p=P)
                wk_r = wkT.ap().rearrange("(co p) d -> p co d", p=P)
                wv_r = wvT.ap().rearrange("(co p) d -> p co d", p=P)

                def project_qk(x_sb, w_sb, h):
                    psq = ps_acc.tile([P, XCH], F32, tag="acc")
                    for c in range(NCO):
                        nc.tensor.matmul(
                            psq[:],
                            w_sb[:, c, h * P : (h + 1) * P],
                            x_sb[:, c, :],
                            start=(c == 0),
                            stop=(c == NCO - 1),
                        )
                    return psq

                def project_v(x_sb, m):
                    psv = ps_acc.tile([P, DL], F32, tag="acc")
                    for c in range(NCO):
                        nc.tensor.matmul(
                            psv[:],
                            x_sb[:, c, m * P : (m + 1) * P],
                            wv_sb[:, c, :],
                            start=(c == 0),
                            stop=(c == NCO - 1),
                        )
                    return psv

                def epilogue_qk(cols, dst_sb, h, scale, psq):
                    # RoPE fully in bf16 SBUF. csA = cos|cos, csB = sin|-sin,
                    # so tmp = [-q2*sin | q1*sin] with base-aligned reads and
                    # the combine is one full-height subtract.
                    qc = pe1.tile([P, XCH], BF16, tag="qc")
                    nc.scalar.copy(qc[:], psq[:])
                    tmp = pe2.tile([P, XCH], BF16, tag="tmp")
                    lo, hi = slice(0, 64), slice(64, P)
                    nc.vector.tensor_tensor(tmp[lo, :], qc[hi, :], csB_sb[hi, cols], MUL)
                    nc.vector.tensor_tensor(tmp[hi, :], qc[lo, :], csB_sb[lo, cols], MUL)
                    qr = pe1.tile([P, XCH], BF16, tag="qr")
                    nc.vector.tensor_tensor(qr[:], qc[:], csA_sb[:, cols], MUL)
                    nc.vector.tensor_tensor(qr[:], qr[:], tmp[:], SUB)
                    # RMS: ssq broadcast over partitions via all-ones lhsT;
                    # rinv = sqrt(scale / ssq) via DVE recip + ACT sqrt
                    q2t = pe2.tile([P, XCH], BF16, tag="q2t")
                    nc.vector.tensor_tensor(q2t[:], qr[:], qr[:], MUL)
                    ssq = ps_ssq.tile([P, XCH], F32, tag="ssq")
                    nc.tensor.matmul(ssq[:], ones_sb[:], q2t[:], start=True, stop=True)
                    r1 = pe2.tile([P, XCH], F32, tag="r1")
                    nc.vector.reciprocal_approx_fast(r1[:], ssq[:])
                    rinv = pe2.tile([P, XCH], BF16, tag="rinv")
                    nc.scalar.activation(rinv[:], r1[:], SQRT, scale=scale)
                    nc.vector.tensor_tensor(dst_sb[:, h, cols], qr[:], rinv[:], MUL)

                def epilogue_v(tcx, m, psv):
                    nc.scalar.copy(V_sb[:, tcx * (XCH // P) + m, :], psv[:])

                for tcx in range(NXCH):
                    cols = slice(tcx * XCH, (tcx + 1) * XCH)
                    x_sb = px.tile([P, NCO, XCH], BF16, tag="x")
                    if tcx == 0:
                        # interleave first-chunk DMAs so the first matmuls
                        # (wq head 0 x chunk 0, low c) start early
                        nc.sync.dma_start(wq_sb[:, 0:4, :], wq_r[:, 0:4, :])
                        nc.sync.dma_start(x_sb[:, 0:4, :], xT_r[:, 0:4, cols])
                        nc.sync.dma_start(ones_sb[:], ones_d.ap())
                        nc.sync.dma_start(csA_sb[:], csA_d.ap())
                        nc.sync.dma_start(csB_sb[:], csB_d.ap())
                        nc.sync.dma_start(wq_sb[:, 4:8, :], wq_r[:, 4:8, :])
                        nc.sync.dma_start(x_sb[:, 4:8, :], xT_r[:, 4:8, cols])
                        nc.sync.dma_start(wq_sb[:, 8:, :], wq_r[:, 8:, :])
                        nc.sync.dma_start(x_sb[:, 8:, :], xT_r[:, 8:, cols])
                        nc.sync.dma_start(wk_sb[:], wk_r[:])
                        nc.sync.dma_start(wv_sb[:], wv_r[:])
                        nc.sync.dma_start(
                            wp_sb[:], wpT.ap().rearrange("(h p) j -> p h j", p=P)
                        )
                        nc.sync.dma_start(tri4_sb[:], tri_d.ap())
                    else:
                        nc.sync.dma_start(x_sb[:], xT_r[:, :, cols])

                    # software-pipeline: unit u's epilogue is interleaved
                    # behind unit u+1's projection matmuls
                    units = (
                        [("q", h) for h in range(HL)]
                        + [("k", h) for h in range(HL)]
                        + [("v", m) for m in range(XCH // P)]
                    )
                    prev = None
                    for kind, idx in units:
                        if kind == "q":
                            ps = project_qk(x_sb, wq_sb, idx)
                        elif kind == "k":
                            ps = project_qk(x_sb, wk_sb, idx)
                        else:
                            ps = project_v(x_sb, idx)
                        if prev is not None:
                            pkind, pidx, pps = prev
                            if pkind == "q":
                                epilogue_qk(cols, QT_sb, pidx, 1.0, pps)
                            elif pkind == "k":
                                epilogue_qk(cols, KT_sb, pidx, float(P), pps)
                            else:
                                epilogue_v(tcx, pidx, pps)
                        prev = (kind, idx, ps)
                    pkind, pidx, pps = prev
                    if pkind == "v":
                        epilogue_v(tcx, pidx, pps)

            # ---- phase D: attention + fused out-projection ------------
            F32R = mybir.dt.float32r
            ADD = mybir.AluOpType.add
            with (
                tc.tile_pool(name="de", bufs=6) as de,
                tc.tile_pool(name="de4", bufs=2) as de4,
                tc.tile_pool(name="de4f", bufs=2) as de4f,
                tc.tile_pool(name="dsum", bufs=2) as dsum,
                tc.tile_pool(name="dm", bufs=2) as dm,
                tc.tile_pool(name="ps_st", bufs=3, space="PSUM") as ps_st,
                tc.tile_pool(name="ps_ot", bufs=2, space="PSUM") as ps_ot,
                tc.tile_pool(name="ps_den", bufs=1, space="PSUM") as ps_den,
                tc.tile_pool(name="ps_po", bufs=2, space="PSUM") as ps_po,
            ):
                ones32 = de.tile([P, P], F32R, tag="ones32")
                nc.sync.dma_start(ones32[:], ones32_d.ap())
                for j in range(NQCH):
                    ot_ch = dm.tile([P, HL, QCH], BF16, tag="otch")
                    jq = slice(j * QCH, (j + 1) * QCH)
                    ntk = (j + 1) * NSTR
                    for h in range(HL):
                        ot_ps = ps_ot.tile([P, QCH], F32, tag="ot")
                        # e4 holds the 4 diagonal-band tiles, computed only on
                        # their causal columns [128r:]; the mask multiply runs
                        # full width (stale leading columns are zeroed by tri4,
                        # which also lets the esum adds run full width)
                        e4 = de4.tile([P, NSTR, QCH], BF16, tag="e4")
                        e4f = de4f.tile([P, NSTR, QCH], BF16, tag="e4f")
                        for r in range(NSTR):
                            i = j * NSTR + r
                            cc = slice(r * P, QCH)
                            st = ps_st.tile([P, QCH], F32, tag="st")
                            nc.tensor.matmul(
                                st[:, cc],
                                KT_sb[:, h, i * P : (i + 1) * P],
                                QT_sb[:, h, j * QCH + r * P : (j + 1) * QCH],
                                start=True,
                                stop=True,
                            )
                            nc.scalar.activation(e4f[:, r, cc], st[:, cc], EXP)
                            # mask per band so the never-written leading
                            # columns of e4f are never read
                            nc.vector.tensor_tensor(
                                e4[:, r, cc], e4f[:, r, cc], tri4_sb[:, r, cc], MUL
                            )
                        # softmax denominator: accumulate e tiles on DVE
                        # (fp32), one all-ones matmul per (j,h) broadcasts the
                        # column sums. The early-ready diag bands seed the sum
                        # (causal columns only); off-diag tiles are added as
                        # their exps land, so the accumulation chain finishes
                        # well before the PE reaches the ones-matmul.
                        esum = dsum.tile([P, QCH], F32R, tag="esum")
                        nc.vector.tensor_copy(esum[:], e4[:, 0, :])
                        for r in range(1, NSTR):
                            cc = slice(r * P, QCH)
                            nc.vector.tensor_tensor(
                                esum[:, cc], esum[:, cc], e4[:, r, cc], ADD
                            )
                        for i in range(ntk):
                            r = i - j * NSTR
                            if r >= 0:
                                cc = slice(r * P, QCH)
                                e_ap = e4[:, r, cc]
                                ocols = cc
                            else:
                                st = ps_st.tile([P, QCH], F32, tag="st")
                                nc.tensor.matmul(
                                    st[:],
                                    KT_sb[:, h, i * P : (i + 1) * P],
                                    QT_sb[:, h, jq],
                                    start=True,
                                    stop=True,
                                )
                                e_sb = de.tile([P, QCH], BF16, tag="e")
                                nc.scalar.activation(e_sb[:], st[:], EXP)
                                e_ap = e_sb[:]
                                ocols = slice(0, QCH)
                                nc.vector.tensor_tensor(
                                    esum[:], esum[:], e_sb[:], ADD
                                )
                            nc.tensor.matmul(
                                ot_ps[:, ocols],
                                V_sb[:, i, h * P : (h + 1) * P],
                                e_ap,
                                start=(i == 0),
                                stop=(i == ntk - 1),
                            )
                        den_ps = ps_den.tile([P, QCH], F32, tag="den")
                        nc.tensor.matmul(
                            den_ps[:], ones32[:], esum[:], start=True, stop=True
                        )
                        recip = dm.tile([P, QCH], F32, tag="recip")
                        nc.vector.reciprocal_approx_fast(recip[:], den_ps[:])
                        nc.vector.tensor_tensor(
                            ot_ch[:, h, :], ot_ps[:], recip[:], MUL
                        )
                    # fused out-projection for tq-chunk j
                    for u in range(QCH // P):
                        for jc in range(NQCH):
                            po = ps_po.tile([P, QCH], F32, tag="po")
                            for h in range(HL):
                                nc.tensor.matmul(
                                    po[:],
                                    ot_ch[:, h, u * P : (u + 1) * P],
                                    wp_sb[:, h, jc * QCH : (jc + 1) * QCH],
                                    start=(h == 0),
                                    stop=(h == HL - 1),
                                )
                            osb = dm.tile([P, QCH], BF16, tag="osb")
                            nc.scalar.copy(osb[:], po[:])
                            nc.sync.dma_start(
                                out_p.ap()[
                                    j * QCH + u * P : j * QCH + (u + 1) * P,
                                    jc * QCH : (jc + 1) * QCH,
                                ],
                                osb[:],
                            )

    nc.compile()
    return nc


_NC = None


def _get_nc():
    global _NC
    if _NC is None:
        _NC = build_program()
    return _NC


def _host_inputs(x, cos, sin, wq, wk, wv, wproj):
    BF = ml_dtypes.bfloat16
    B = x.shape[0]
    cosT = np.ascontiguousarray(cos[0, :, 0, :].T).astype(np.float32)  # [64, T]
    sinT = np.ascontiguousarray(sin[0, :, 0, :].T).astype(np.float32)
    csA = np.concatenate([cosT, cosT], axis=0).astype(BF)
    csB = np.concatenate([sinT, -sinT], axis=0).astype(BF)
    # tri[p, r, f] = 1 iff causal (tk=128r+p <= tq=f) within a diagonal band
    rr, pp, ff = np.meshgrid(np.arange(NSTR), np.arange(P), np.arange(QCH), indexing="ij")
    tri = np.ascontiguousarray(
        (pp + 128 * rr <= ff).astype(np.float32).transpose(1, 0, 2)
    ).astype(BF)
    ones = np.ones((P, P), BF)
    ones32f = np.ones((P, P), np.float32)

    xTs = [np.ascontiguousarray(x[b].T).astype(BF) for b in range(B)]
    in_maps = []
    for core in range(8):
        b, g = divmod(core, 4)
        sl = slice(g * DL, (g + 1) * DL)
        in_maps.append({
            "xT": xTs[b],
            "wqT": np.ascontiguousarray(wq[sl, :].T).astype(BF),
            "wkT": np.ascontiguousarray(wk[sl, :].T).astype(BF),
            "wvT": np.ascontiguousarray(wv[sl, :].T).astype(BF),
            "wpT": np.ascontiguousarray(wproj[:, sl].T).astype(BF),
            "csA": csA, "csB": csB, "tri": tri, "ones": ones,
            "ones32": ones32f,
        })
    return in_maps


def kernel(x, cos, sin, wq, wk, wv, wproj, _trace=False):
    nc = _get_nc()
    in_maps = _host_inputs(x, cos, sin, wq, wk, wv, wproj)
    res = run_bass_kernel_spmd(nc, in_maps, core_ids=list(range(8)), trace=_trace)
    parts = [res.results[c]["out_p"].astype(np.float32) for c in range(8)]
    out = np.stack([
        sum(parts[0:4]),
        sum(parts[4:8]),
    ]).astype(np.float32)
    kernel.last_exec_time_ns = res.exec_time_ns
    kernel.last_result = res
    return out


# revision 18
# speedup vs baseline: 1.4886x; 1.1641x over previous
"""Causal self-attention (RoPE + RMS-norm QK, 16 heads) on 8 Trainium2 cores.

Sharding: core c = (b, g) with b = c // 4 (batch), g = c % 4 (head group of 4).
Each core computes q/k/v projections for its 4 heads from x[b], runs causal
attention, and the out-projection restricted to its head-group columns of
wproj; the host sums the 4 partial outputs per batch.

v2 layout vs the fp32r baseline:
- everything bf16 on the input side (same PE rate as fp32r, half the DMA and
  SBUF traffic, 2-4x DVE throughput); fp32 only inside PSUM accumulation.
- single pass over x: Q, K and V projections for each x t-chunk, so x is
  DMA'd once (bf16) instead of three times (fp32).
- all weights prefetched behind the first chunk's compute; DMAs ordered so
  the first matmul can start after ~2 MB of transfers.
- rms-norm uses DVE reciprocal_approx_fast + ACT Sqrt, so phase P only ever
  needs the sqrt act table and phase D the exp table: 2 table loads total
  (the Ln/Exp alternation in the baseline caused 65 table loads, 83us).
- bf16 output (host sums partials in fp32), halving the output DMA.

Per-core layout ("transposed-S"): projections produce Q^T/K^T with head-dim
on partitions, V in natural [t, d] layout. Scores are computed transposed
(S^T[tk, tq]) so softmax needs no transposes: exp only (logits are bounded
by sqrt(D) after RMS-norm), the denominator comes from an all-ones lhsT
matmul that broadcasts column sums across partitions, and the divide happens
on the P.V result's move out of PSUM. The out-projection is fused per
tq-chunk.
"""

import numpy as np
import ml_dtypes

import concourse.bass as bass
import concourse.mybir as mybir
import concourse.tile as tile
from concourse import bacc
from concourse.bass_utils import run_bass_kernel_spmd

P = 128          # partitions / head dim
T = 2048         # sequence length
C = 2048         # model dim
HL = 4           # heads per core
DL = HL * P      # local projection width (512)
NCO = C // P     # c-chunks (16)
XCH = 512        # x t-chunk width for projections
NXCH = T // XCH  # 4
QCH = 512        # tq-chunk width for attention
NQCH = T // QCH  # 4
NSTR = QCH // P  # diagonal-band tiles per chunk (4)
NTT = T // P     # t-tiles (16)

F32 = mybir.dt.float32
BF16 = mybir.dt.bfloat16
MUL = mybir.AluOpType.mult
SUB = mybir.AluOpType.subtract
SQRT = mybir.ActivationFunctionType.Sqrt
EXP = mybir.ActivationFunctionType.Exp


def build_program():
    nc = bacc.Bacc("TRN2", target_bir_lowering=False, debug=False, num_devices=8)

    xT = nc.dram_tensor("xT", [C, T], BF16, kind="ExternalInput")
    wqT = nc.dram_tensor("wqT", [C, DL], BF16, kind="ExternalInput")
    wkT = nc.dram_tensor("wkT", [C, DL], BF16, kind="ExternalInput")
    wvT = nc.dram_tensor("wvT", [C, DL], BF16, kind="ExternalInput")
    wpT = nc.dram_tensor("wpT", [DL, C], BF16, kind="ExternalInput")
    csA_d = nc.dram_tensor("csA", [P, T], BF16, kind="ExternalInput")   # cos|cos
    csB_d = nc.dram_tensor("csB", [P, T], BF16, kind="ExternalInput")   # sin|-sin
    tri_d = nc.dram_tensor("tri", [P, NSTR, QCH], BF16, kind="ExternalInput")
    ones_d = nc.dram_tensor("ones", [P, P], BF16, kind="ExternalInput")
    out_p = nc.dram_tensor("out_p", [T, C], BF16, kind="ExternalOutput")

    xT_r = xT.ap().rearrange("(co p) t -> p co t", p=P)

    with tile.TileContext(nc) as tc:
        with tc.tile_pool(name="base", bufs=1) as base:
            QT_sb = base.tile([P, HL, T], BF16, tag="QT")   # [d, h, tq]
            KT_sb = base.tile([P, HL, T], BF16, tag="KT")   # [d, h, tk]
            V_sb = base.tile([P, NTT, DL], BF16, tag="V")   # [t_sub, t_tile, d]
            ones_sb = base.tile([P, P], BF16, tag="ones")
            csA_sb = base.tile([P, T], BF16, tag="csA")
            csB_sb = base.tile([P, T], BF16, tag="csB")
            wp_sb = base.tile([P, HL, C], BF16, tag="wp")
            tri4_sb = base.tile([P, NSTR, QCH], BF16, tag="tri4")

            # ---- phase P: Q/K/V projections in one pass over x --------
            with (
                tc.tile_pool(name="pw", bufs=1) as pw,
                tc.tile_pool(name="px", bufs=2) as px,
                tc.tile_pool(name="pe1", bufs=2) as pe1,
                tc.tile_pool(name="pe2", bufs=2) as pe2,
                tc.tile_pool(name="ps_acc", bufs=3, space="PSUM") as ps_acc,
                tc.tile_pool(name="ps_ssq", bufs=2, space="PSUM") as ps_ssq,
            ):
                wq_sb = pw.tile([P, NCO, DL], BF16, tag="wq")
                wk_sb = pw.tile([P, NCO, DL], BF16, tag="wk")
                wv_sb = pw.tile([P, NCO, DL], BF16, tag="wv")
                wq_r = wqT.ap().rearrange("(co p) d -> p co d", p=P)
                wk_r = wkT.ap().rearrange("(co p) d -> p co d", p=P)
                wv_r = wvT.ap().rearrange("(co p) d -> p co d", p=P)

                def project_qk(x_sb, w_sb, h):
                    psq = ps_acc.tile([P, XCH], F32, tag="acc")
                    for c in range(NCO):
                        nc.tensor.matmul(
                            psq[:],
                            w_sb[:, c, h * P : (h + 1) * P],
                            x_sb[:, c, :],
                            start=(c == 0),
                            stop=(c == NCO - 1),
                        )
                    return psq

                def project_v(x_sb, m):
                    psv = ps_acc.tile([P, DL], F32, tag="acc")
                    for c in range(NCO):
                        nc.tensor.matmul(
                            psv[:],
                            x_sb[:, c, m * P : (m + 1) * P],
                            wv_sb[:, c, :],
                            start=(c == 0),
                            stop=(c == NCO - 1),
                        )
                    return psv

                def epilogue_qk(cols, dst_sb, h, scale, psq):
                    # RoPE fully in bf16 SBUF. csA = cos|cos, csB = sin|-sin,
                    # so tmp = [-q2*sin | q1*sin] with base-aligned reads and
                    # the combine is one full-height subtract.
                    qc = pe1.tile([P, XCH], BF16, tag="qc")
                    nc.scalar.copy(qc[:], psq[:])
                    tmp = pe2.tile([P, XCH], BF16, tag="tmp")
                    lo, hi = slice(0, 64), slice(64, P)
                    nc.vector.tensor_tensor(tmp[lo, :], qc[hi, :], csB_sb[hi, cols], MUL)
                    nc.vector.tensor_tensor(tmp[hi, :], qc[lo, :], csB_sb[lo, cols], MUL)
                    qr = pe1.tile([P, XCH], BF16, tag="qr")
                    nc.vector.tensor_tensor(qr[:], qc[:], csA_sb[:, cols], MUL)
                    nc.vector.tensor_tensor(qr[:], qr[:], tmp[:], SUB)
                    # RMS: ssq broadcast over partitions via all-ones lhsT;
                    # rinv = sqrt(scale / ssq) via DVE recip + ACT sqrt
                    q2t = pe2.tile([P, XCH], BF16, tag="q2t")
                    nc.vector.tensor_tensor(q2t[:], qr[:], qr[:], MUL)
                    ssq = ps_ssq.tile([P, XCH], F32, tag="ssq")
                    nc.tensor.matmul(ssq[:], ones_sb[:], q2t[:], start=True, stop=True)
                    r1 = pe2.tile([P, XCH], F32, tag="r1")
                    nc.vector.reciprocal_approx_fast(r1[:], ssq[:])
                    rinv = pe2.tile([P, XCH], BF16, tag="rinv")
                    nc.scalar.activation(rinv[:], r1[:], SQRT, scale=scale)
                    nc.vector.tensor_tensor(dst_sb[:, h, cols], qr[:], rinv[:], MUL)

                def epilogue_v(tcx, m, psv):
                    nc.scalar.copy(V_sb[:, tcx * (XCH // P) + m, :], psv[:])

                for tcx in range(NXCH):
                    cols = slice(tcx * XCH, (tcx + 1) * XCH)
                    x_sb = px.tile([P, NCO, XCH], BF16, tag="x")
                    if tcx == 0:
                        # interleave first-chunk DMAs so the first matmuls
                        # (wq head 0 x chunk 0, low c) start early
                        nc.sync.dma_start(wq_sb[:, 0:4, :], wq_r[:, 0:4, :])
                        nc.sync.dma_start(x_sb[:, 0:4, :], xT_r[:, 0:4, cols])
                        nc.sync.dma_start(ones_sb[:], ones_d.ap())
                        nc.sync.dma_start(csA_sb[:], csA_d.ap())
                        nc.sync.dma_start(csB_sb[:], csB_d.ap())
                        nc.sync.dma_start(wq_sb[:, 4:8, :], wq_r[:, 4:8, :])
                        nc.sync.dma_start(x_sb[:, 4:8, :], xT_r[:, 4:8, cols])
                        nc.sync.dma_start(wq_sb[:, 8:, :], wq_r[:, 8:, :])
                        nc.sync.dma_start(x_sb[:, 8:, :], xT_r[:, 8:, cols])
                        nc.sync.dma_start(wk_sb[:], wk_r[:])
                        nc.sync.dma_start(wv_sb[:], wv_r[:])
                        nc.sync.dma_start(
                            wp_sb[:], wpT.ap().rearrange("(h p) j -> p h j", p=P)
                        )
                        nc.sync.dma_start(tri4_sb[:], tri_d.ap())
                    else:
                        nc.sync.dma_start(x_sb[:], xT_r[:, :, cols])

                    # software-pipeline: unit u's epilogue is interleaved
                    # behind unit u+1's projection matmuls
                    units = (
                        [("q", h) for h in range(HL)]
                        + [("k", h) for h in range(HL)]
                        + [("v", m) for m in range(XCH // P)]
                    )
                    prev = None
                    for kind, idx in units:
                        if kind == "q":
                            ps = project_qk(x_sb, wq_sb, idx)
                        elif kind == "k":
                            ps = project_qk(x_sb, wk_sb, idx)
                        else:
                            ps = project_v(x_sb, idx)
                        if prev is not None:
                            pkind, pidx, pps = prev
                            if pkind == "q":
                                epilogue_qk(cols, QT_sb, pidx, 1.0, pps)
                            elif pkind == "k":
                                epilogue_qk(cols, KT_sb, pidx, float(P), pps)
                            else:
                                epilogue_v(tcx, pidx, pps)
                        prev = (kind, idx, ps)
                    pkind, pidx, pps = prev
                    if pkind == "v":
                        epilogue_v(tcx, pidx, pps)

            # ---- phase D: attention + fused out-projection ------------
            F32R = mybir.dt.float32r
            ADD = mybir.AluOpType.add
            with (
                tc.tile_pool(name="de", bufs=6) as de,
                tc.tile_pool(name="de4", bufs=2) as de4,
                tc.tile_pool(name="de4f", bufs=2) as de4f,
                tc.tile_pool(name="dsum", bufs=2) as dsum,
                tc.tile_pool(name="dm", bufs=2) as dm,
                tc.tile_pool(name="ps_st", bufs=3, space="PSUM") as ps_st,
                tc.tile_pool(name="ps_ot", bufs=2, space="PSUM") as ps_ot,
                tc.tile_pool(name="ps_den", bufs=1, space="PSUM") as ps_den,
                tc.tile_pool(name="ps_po", bufs=2, space="PSUM") as ps_po,
            ):

                for j in range(NQCH):
                    ot_ch = dm.tile([P, HL, QCH], BF16, tag="otch")
                    jq = slice(j * QCH, (j + 1) * QCH)
                    ntk = (j + 1) * NSTR
                    for h in range(HL):
                        ot_ps = ps_ot.tile([P, QCH], F32, tag="ot")
                        # e4 holds the 4 diagonal-band tiles, computed only on
                        # their causal columns [128r:]; the mask multiply runs
                        # full width (stale leading columns are zeroed by tri4,
                        # which also lets the esum adds run full width)
                        e4 = de4.tile([P, NSTR, QCH], BF16, tag="e4")
                        e4f = de4f.tile([P, NSTR, QCH], BF16, tag="e4f")
                        for r in range(NSTR):
                            i = j * NSTR + r
                            cc = slice(r * P, QCH)
                            st = ps_st.tile([P, QCH], F32, tag="st")
                            nc.tensor.matmul(
                                st[:, cc],
                                KT_sb[:, h, i * P : (i + 1) * P],
                                QT_sb[:, h, j * QCH + r * P : (j + 1) * QCH],
                                start=True,
                                stop=True,
                            )
                            nc.scalar.activation(e4f[:, r, cc], st[:, cc], EXP)
                            # mask per band so the never-written leading
                            # columns of e4f are never read. For j>=2 the
                            # masked tiles aren't needed until late in the
                            # i-loop, so the idle GpSimd can do them.
                            meng = nc.gpsimd if j >= 2 else nc.vector
                            meng.tensor_tensor(
                                e4[:, r, cc], e4f[:, r, cc], tri4_sb[:, r, cc], MUL
                            )
                        # softmax denominator: accumulate e tiles on DVE in
                        # bf16 (numerically validated on the host), one
                        # all-ones bf16 matmul per (j,h) broadcasts the column
                        # sums in fp32. The early-ready diag bands seed the
                        # sum; off-diag tiles are pair-combined first so the
                        # serial chain is half as long and finishes well
                        # before the PE reaches the ones-matmul.
                        esum = dsum.tile([P, QCH], BF16, tag="esum")
                        nc.vector.tensor_copy(esum[:], e4[:, 0, :])
                        for r in range(1, NSTR):
                            cc = slice(r * P, QCH)
                            nc.vector.tensor_tensor(
                                esum[:, cc], esum[:, cc], e4[:, r, cc], ADD
                            )
                        pend = []

                        def esum_add(e_new):
                            # pair-combine on the idle GpSimd (parallel,
                            # off the serial chain), chain-add on DVE
                            pend.append(e_new)
                            if len(pend) == 2:
                                pair = dsum.tile([P, QCH], BF16, tag="pair")
                                nc.gpsimd.tensor_tensor(
                                    pair[:], pend[0], pend[1], ADD
                                )
                                nc.vector.tensor_tensor(
                                    esum[:], esum[:], pair[:], ADD
                                )
                                pend.clear()

                        for i in range(ntk):
                            r = i - j * NSTR
                            if r >= 0:
                                cc = slice(r * P, QCH)
                                e_ap = e4[:, r, cc]
                                ocols = cc
                            else:
                                st = ps_st.tile([P, QCH], F32, tag="st")
                                nc.tensor.matmul(
                                    st[:],
                                    KT_sb[:, h, i * P : (i + 1) * P],
                                    QT_sb[:, h, jq],
                                    start=True,
                                    stop=True,
                                )
                                e_sb = de.tile([P, QCH], BF16, tag="e")
                                nc.scalar.activation(e_sb[:], st[:], EXP)
                                e_ap = e_sb[:]
                                ocols = slice(0, QCH)
                                esum_add(e_sb[:])
                            nc.tensor.matmul(
                                ot_ps[:, ocols],
                                V_sb[:, i, h * P : (h + 1) * P],
                                e_ap,
                                start=(i == 0),
                                stop=(i == ntk - 1),
                            )
                        if pend:
                            nc.vector.tensor_tensor(
                                esum[:], esum[:], pend[0], ADD
                            )
                            pend.clear()
                        den_ps = ps_den.tile([P, QCH], F32, tag="den")
                        nc.tensor.matmul(
                            den_ps[:], ones_sb[:], esum[:], start=True, stop=True
                        )
                        recip = dm.tile([P, QCH], F32, tag="recip")
                        nc.vector.reciprocal_approx_fast(recip[:], den_ps[:])
                        nc.vector.tensor_tensor(
                            ot_ch[:, h, :], ot_ps[:], recip[:], MUL
                        )
                    # fused out-projection for tq-chunk j
                    for u in range(QCH // P):
                        for jc in range(NQCH):
                            po = ps_po.tile([P, QCH], F32, tag="po")
                            for h in range(HL):
                                nc.tensor.matmul(
                                    po[:],
                                    ot_ch[:, h, u * P : (u + 1) * P],
                                    wp_sb[:, h, jc * QCH : (jc + 1) * QCH],
                                    start=(h == 0),
                                    stop=(h == HL - 1),
                                )
                            osb = dm.tile([P, QCH], BF16, tag="osb")
                            nc.scalar.copy(osb[:], po[:])
                            nc.sync.dma_start(
                                out_p.ap()[
                                    j * QCH + u * P : j * QCH + (u + 1) * P,
                                    jc * QCH : (jc + 1) * QCH,
                                ],
                                osb[:],
                            )

    nc.compile()
    return nc


_NC = None


def _get_nc():
    global _NC
    if _NC is None:
        _NC = build_program()
    return _NC


def _host_inputs(x, cos, sin, wq, wk, wv, wproj):
    BF = ml_dtypes.bfloat16
    B = x.shape[0]
    cosT = np.ascontiguousarray(cos[0, :, 0, :].T).astype(np.float32)  # [64, T]
    sinT = np.ascontiguousarray(sin[0, :, 0, :].T).astype(np.float32)
    csA = np.concatenate([cosT, cosT], axis=0).astype(BF)
    csB = np.concatenate([sinT, -sinT], axis=0).astype(BF)
    # tri[p, r, f] = 1 iff causal (tk=128r+p <= tq=f) within a diagonal band
    rr, pp, ff = np.meshgrid(np.arange(NSTR), np.arange(P), np.arange(QCH), indexing="ij")
    tri = np.ascontiguousarray(
        (pp + 128 * rr <= ff).astype(np.float32).transpose(1, 0, 2)
    ).astype(BF)
    ones = np.ones((P, P), BF)

    xTs = [np.ascontiguousarray(x[b].T).astype(BF) for b in range(B)]
    in_maps = []
    for core in range(8):
        b, g = divmod(core, 4)
        sl = slice(g * DL, (g + 1) * DL)
        in_maps.append({
            "xT": xTs[b],
            "wqT": np.ascontiguousarray(wq[sl, :].T).astype(BF),
            "wkT": np.ascontiguousarray(wk[sl, :].T).astype(BF),
            "wvT": np.ascontiguousarray(wv[sl, :].T).astype(BF),
            "wpT": np.ascontiguousarray(wproj[:, sl].T).astype(BF),
            "csA": csA, "csB": csB, "tri": tri, "ones": ones,
        })
    return in_maps


def kernel(x, cos, sin, wq, wk, wv, wproj, _trace=False):
    nc = _get_nc()
    in_maps = _host_inputs(x, cos, sin, wq, wk, wv, wproj)
    res = run_bass_kernel_spmd(nc, in_maps, core_ids=list(range(8)), trace=_trace)
    parts = [res.results[c]["out_p"].astype(np.float32) for c in range(8)]
    out = np.stack([
        sum(parts[0:4]),
        sum(parts[4:8]),
    ]).astype(np.float32)
    kernel.last_exec_time_ns = res.exec_time_ns
    kernel.last_result = res
    return out
